# revision 1
# baseline (speedup 1.0000x reference)
"""Trainium2 Bass kernel for nn_ClassLoss (YOLO-style classification CE loss).

Strategy: the loss depends only on grid cells hit by valid target boxes
(<=50 cells/batch out of 4096). Each cell corresponds to 3 consecutive
"flat rows" of the [12288, 85] logits block (765 contiguous floats in DRAM).
So instead of streaming 127MB of logits, each core:
  1. loads its 4 batches' targets,
  2. computes per-box (row, col, class, valid), resolves last-write-wins
     duplicates with a pairwise comparison (block-diagonal across batches),
  3. indirect-DMA-gathers the needed cell blocks (two [100, 255] gathers,
     batches stacked in pairs along the partition axis),
  4. computes logsumexp over the 80 classes for the 3 rows of each cell and
     the label logit via a one-hot dot, masked by the winner flags,
  5. reduces to per-batch (loss_sum, cell_count) pairs via a selector matmul.
Host applies the per-batch mean (num / max(3*cnt,1)), sums across cores and
divides by the global batch size (the all-reduce + normalize of the
data-parallel sharding).
"""

import sys

sys.path.insert(0, "/opt/trn_rl_repo")

import numpy as np

import concourse.bass as bass
import concourse.tile as tile
from concourse import bacc, mybir
from concourse.bass_utils import run_bass_kernel_spmd

# Problem constants (hardcoded per harness contract).
B, A, H, W, NC_CLS, M = 32, 3, 64, 64, 80, 50
N_CORES = 8
B_CORE = B // N_CORES          # 4 batches per core
CELLS = H * W                  # 4096 cells per batch
ROWLEN = 3 * (5 + NC_CLS)      # 255 floats per cell (3 anchor rows x 85)
P2 = 2 * M                     # 100 partitions: 2 batches x 50 boxes
FP32 = mybir.dt.float32
I32 = mybir.dt.int32
Alu = mybir.AluOpType
Act = mybir.ActivationFunctionType


def _host_consts():
    # cidx[*, a*85 + k] = k-5 for k in [5,85), else -1 (never matches a class)
    cidx = np.full((P2, ROWLEN), -1.0, dtype=np.float32)
    for a in range(3):
        cidx[:, a * 85 + 5 : (a + 1) * 85] = np.arange(NC_CLS, dtype=np.float32)
    # ut2[p, q] = 1 iff same 50-block and q%50 > p%50 (strictly-later box)
    blk = np.arange(P2) // M
    mi = np.arange(P2) % M
    ut2 = ((blk[:, None] == blk[None, :]) & (mi[None, :] > mi[:, None])).astype(
        np.float32
    )
    ident = np.eye(P2, dtype=np.float32)
    # cell offset per partition, per pair: batch = 2*j + p//50
    boff = np.empty((P2, 2), dtype=np.float32)
    for j in range(2):
        boff[:M, j] = (2 * j) * CELLS
        boff[M:, j] = (2 * j + 1) * CELLS
    # block selector for per-batch partition sums
    bsel = np.zeros((P2, 2), dtype=np.float32)
    bsel[:M, 0] = 1.0
    bsel[M:, 1] = 1.0
    return {"cidx": cidx, "ut2": ut2, "ident": ident, "boff": boff, "bsel": bsel}


def _build_kernel_body(tc, x_ap, t_ap, out_ap, cidx_ap, ut_ap, ident_ap, boff_ap, bsel_ap):
    nc = tc.nc
    from contextlib import ExitStack

    ctx = ExitStack()
    with ctx:
        consts = ctx.enter_context(tc.tile_pool(name="consts", bufs=1))
        work = ctx.enter_context(tc.tile_pool(name="work", bufs=3))
        gpool = ctx.enter_context(tc.tile_pool(name="gather", bufs=2))
        psum = ctx.enter_context(tc.tile_pool(name="psum", bufs=2, space="PSUM"))
        psumr = ctx.enter_context(tc.tile_pool(name="psumr", bufs=1, space="PSUM"))
        fpool = ctx.enter_context(tc.tile_pool(name="final", bufs=1))

        # ---- constants / persistent tiles ----
        cidx_t = consts.tile([P2, ROWLEN], FP32)
        nc.sync.dma_start(cidx_t[:], cidx_ap[:])
        ut_t = consts.tile([P2, P2], FP32)
        nc.sync.dma_start(ut_t[:], ut_ap[:])
        ident_t = consts.tile([P2, P2], FP32)
        nc.sync.dma_start(ident_t[:], ident_ap[:])
        boff_t = consts.tile([P2, 2], FP32)
        nc.sync.dma_start(boff_t[:], boff_ap[:])
        bsel_t = consts.tile([P2, 2], FP32)
        nc.sync.dma_start(bsel_t[:], bsel_ap[:])

        stats = fpool.tile([P2, 4], FP32)  # (num, cnt) per pair-column

        # all targets: [100, 2, 5]; partition p = batch-in-pair p//50, box p%50
        tgt_t = consts.tile([P2, 2 * 5], FP32)
        nc.sync.dma_start(
            tgt_t[:].rearrange("p (j f) -> p j f", f=5),
            t_ap.rearrange("(j bb) m f -> (bb m) j f", j=2),
        )

        MAGIC = 8388608.0  # 2^23

        for j in range(2):
            Tb = tgt_t[:].rearrange("p (j f) -> p j f", f=5)[:, j, :]
            cls = Tb[:, 0:1]

            # valid[m] = sum(|t|) > 0
            val1 = work.tile([P2, 1], FP32, tag="val1")
            nc.vector.tensor_reduce(
                val1[:], Tb, axis=mybir.AxisListType.X, op=Alu.add,
                apply_absolute_value=True,
            )
            valid = work.tile([P2, 1], FP32, tag="valid")
            nc.vector.tensor_scalar(valid[:], val1[:], 0.0, None, op0=Alu.is_gt)

            # (c, r) = floor((x, y)*64) fused on [100, 2]: exact branchless
            # floor via ri = RNE(v) (magic add/sub), floor = ri - (ri > v)
            v2 = work.tile([P2, 2], FP32, tag="v2")
            nc.vector.tensor_scalar(v2[:], Tb[:, 1:3], 64.0, None, op0=Alu.mult)
            ri2 = work.tile([P2, 2], FP32, tag="ri2")
            nc.vector.tensor_scalar(
                ri2[:], Tb[:, 1:3], 64.0, MAGIC, op0=Alu.mult, op1=Alu.add
            )
            nc.vector.tensor_scalar(ri2[:], ri2[:], MAGIC, None, op0=Alu.subtract)
            corr2 = work.tile([P2, 2], FP32, tag="corr2")
            nc.vector.tensor_tensor(corr2[:], ri2[:], v2[:], op=Alu.is_gt)
            fl2 = work.tile([P2, 2], FP32, tag="fl2")
            nc.vector.tensor_tensor(fl2[:], ri2[:], corr2[:], op=Alu.subtract)
            cc, rr = fl2[:, 0:1], fl2[:, 1:2]

            # cell = r*64 + c + batch_offset
            cellf = work.tile([P2, 1], FP32, tag="cellf")
            nc.vector.scalar_tensor_tensor(
                cellf[:], rr, 64.0, cc, op0=Alu.mult, op1=Alu.add
            )
            celli = work.tile([P2, 1], I32, tag="celli")
            nc.vector.tensor_tensor(
                celli[:], cellf[:], boff_t[:, j : j + 1], op=Alu.add
            )

            # ---- gather the 100 cell blocks [100, 255] ASAP so the DMA and
            # exp overlap the winner resolution below ----
            graw = gpool.tile([P2, ROWLEN], FP32, tag="graw")
            nc.gpsimd.indirect_dma_start(
                out=graw[:],
                out_offset=None,
                in_=x_ap,
                in_offset=bass.IndirectOffsetOnAxis(ap=celli[:, :1], axis=0),
            )
            gv = graw[:].rearrange("p (a f) -> p a f", a=3)[:, :, 5:]
            ex = gpool.tile([P2, 3 * NC_CLS], FP32, tag="ex")
            nc.scalar.activation(
                ex[:].rearrange("p (a f) -> p a f", f=NC_CLS), gv, Act.Exp
            )

            # ---- winner resolution (last valid write wins) ----
            # key = valid ? cell : -1 so invalid boxes never match any cell
            key = work.tile([P2, 1], FP32, tag="key")
            nc.vector.scalar_tensor_tensor(
                key[:], cellf[:], 1.0, valid[:], op0=Alu.add, op1=Alu.mult
            )
            nc.vector.tensor_scalar(key[:], key[:], -1.0, None, op0=Alu.add)

            qT = psum.tile([P2, P2], FP32, tag="qT", space="PSUM")
            nc.tensor.transpose(qT[:], key[:].to_broadcast([P2, P2]), ident_t[:])

            same = work.tile([P2, P2], FP32, tag="same")
            nc.vector.tensor_scalar(same[:], qT[:], key[:], None, op0=Alu.is_equal)
            scrap0 = work.tile([P2, P2], FP32, tag="scrap0")
            coll = work.tile([P2, 1], FP32, tag="coll")
            nc.gpsimd.tensor_tensor(scrap0[:], same[:], ut_t[:], op=Alu.mult)
            nc.vector.tensor_reduce(
                coll[:], scrap0[:], axis=mybir.AxisListType.X, op=Alu.add
            )
            winner = work.tile([P2, 1], FP32, tag="winner")
            nc.vector.scalar_tensor_tensor(
                winner[:], coll[:], 0.0, valid[:], op0=Alu.is_equal, op1=Alu.mult
            )

            # ---- per-cell CE pieces ----
            se = work.tile([P2, 3], FP32, tag="se")
            nc.vector.tensor_reduce(
                se[:], ex[:].rearrange("p (a f) -> p a f", f=NC_CLS),
                axis=mybir.AxisListType.X, op=Alu.add,
            )
            lse = work.tile([P2, 3], FP32, tag="lse")
            nc.scalar.activation(lse[:], se[:], Act.Ln)
            s3 = work.tile([P2, 1], FP32, tag="s3")
            nc.vector.tensor_reduce(
                s3[:], lse[:], axis=mybir.AxisListType.X, op=Alu.add
            )

            # label logit sum over the 3 rows: one-hot dot against cidx
            ohc = work.tile([P2, ROWLEN], FP32, tag="ohc")
            nc.gpsimd.tensor_scalar(ohc[:], cidx_t[:], cls, None, op0=Alu.is_equal)
            scrap1 = work.tile([P2, ROWLEN], FP32, tag="scrap1")
            nc.gpsimd.tensor_tensor(scrap1[:], ohc[:], graw[:], op=Alu.mult)
            g3 = work.tile([P2, 1], FP32, tag="g3")
            nc.vector.tensor_reduce(
                g3[:], scrap1[:], axis=mybir.AxisListType.X, op=Alu.add
            )

            # d = (lse_sum - label_logit_sum); stats cols: num = winner*d, cnt = winner
            d = work.tile([P2, 1], FP32, tag="d")
            nc.vector.tensor_tensor(d[:], s3[:], g3[:], op=Alu.subtract)
            nc.vector.tensor_tensor(
                stats[:, 2 * j : 2 * j + 1], d[:], winner[:], op=Alu.mult
            )
            nc.vector.tensor_copy(stats[:, 2 * j + 1 : 2 * j + 2], winner[:])

        # ---- per-batch partition sums via PE: red[i, 2j+k] = batch 2j+i ----
        red = psumr.tile([2, 4], FP32, tag="red", space="PSUM")
        nc.tensor.matmul(red[:], bsel_t[:], stats[:], start=True, stop=True)
        fin = fpool.tile([2, 4], FP32)
        nc.vector.tensor_copy(fin[:], red[:])
        nc.sync.dma_start(out_ap[:], fin[:])


_CACHE = {}


def _get_compiled():
    if "nc" in _CACHE:
        return _CACHE["nc"]
    nc = bacc.Bacc(
        "TRN2",
        target_bir_lowering=False,
        debug=False,
        enable_asserts=False,
        num_devices=N_CORES,
    )
    x = nc.dram_tensor("xflat", [B_CORE * CELLS, ROWLEN], FP32, kind="ExternalInput")
    t = nc.dram_tensor("tgt", [B_CORE, M, 5], FP32, kind="ExternalInput")
    cidx = nc.dram_tensor("cidx", [P2, ROWLEN], FP32, kind="ExternalInput")
    ut2 = nc.dram_tensor("ut2", [P2, P2], FP32, kind="ExternalInput")
    ident = nc.dram_tensor("ident", [P2, P2], FP32, kind="ExternalInput")
    boff = nc.dram_tensor("boff", [P2, 2], FP32, kind="ExternalInput")
    bsel = nc.dram_tensor("bsel", [P2, 2], FP32, kind="ExternalInput")
    out = nc.dram_tensor("statsout", [2, 4], FP32, kind="ExternalOutput")

    with tile.TileContext(nc) as tc:
        _build_kernel_body(
            tc, x.ap(), t.ap(), out.ap(), cidx.ap(), ut2.ap(), ident.ap(),
            boff.ap(), bsel.ap(),
        )
    nc.compile()
    _CACHE["nc"] = nc
    return nc


def _finish(stats_list):
    """Host: per-batch mean, then mean over global batch (float64)."""
    total = 0.0
    for st in stats_list:
        st = np.asarray(st, dtype=np.float64)  # [2, 4]
        for j in range(2):
            for i in range(2):
                num = st[i, 2 * j]
                cnt = st[i, 2 * j + 1]
                total += num / max(3.0 * cnt, 1.0)
    return total / B


def _run(output, targets, trace=False):
    nc = _get_compiled()
    consts = _host_consts()
    output = np.ascontiguousarray(output, dtype=np.float32)
    targets = np.ascontiguousarray(targets, dtype=np.float32)
    in_maps = []
    for k in range(N_CORES):
        in_maps.append(
            {
                "xflat": output[k * B_CORE : (k + 1) * B_CORE].reshape(
                    B_CORE * CELLS, ROWLEN
                ),
                "tgt": targets[k * B_CORE : (k + 1) * B_CORE],
                **consts,
            }
        )
    res = run_bass_kernel_spmd(nc, in_maps, core_ids=list(range(N_CORES)), trace=trace)
    total = _finish([r["statsout"] for r in res.results])
    return np.float32(total), res


def kernel(output, targets):
    val, _ = _run(output, targets)
    return np.asarray(val, dtype=np.float32)



# revision 5
# speedup vs baseline: 1.5226x; 1.5226x over previous
"""Trainium2 Bass kernel for nn_ClassLoss (YOLO-style classification CE loss).

Strategy: the loss depends only on grid cells hit by valid target boxes
(<=50 cells/batch out of 4096). Each cell corresponds to 3 consecutive
"flat rows" of the [12288, 85] logits block (765 contiguous floats in DRAM).
So instead of streaming 127MB of logits, each core:
  1. loads its 4 batches' targets as [100, 10] (partition = 2-batch-pair x
     50 boxes, free = pair-slot j x 5 fields) plus one packed const tensor,
  2. computes per-box (cell, class, valid) for both pair-slots at once on
     [100, 2] tiles (branchless floor via the 2^23 magic constant),
  3. indirect-DMA-gathers the two [100, 255] cell blocks into one tile,
  4. resolves last-write-wins duplicates with a PE transpose + fused
     (is_equal * upper-tri, accum) scalar_tensor_tensor per pair-slot,
  5. computes per-(box, anchor) softmax denominators with 6 Exp activations
     (accum_out = row sum) and the label-logit sums with fused
     (cidx == cls) * graw accumulating stts,
  6. DMAs the per-box stats [100, 10] = (se x 6, winner x 2, g3 x 2).
Host finishes in float64: d = sum_a ln(se) - g3, per-batch mean
num / max(3*cnt, 1), sum over batches / B (the data-parallel all-reduce).
"""

import sys

sys.path.insert(0, "/opt/trn_rl_repo")

import numpy as np

import concourse.bass as bass
import concourse.tile as tile
from concourse import bacc, mybir
from concourse.bass_utils import run_bass_kernel_spmd

# Problem constants (hardcoded per harness contract).
B, A, H, W, NC_CLS, M = 32, 3, 64, 64, 80, 50
N_CORES = 8
B_CORE = B // N_CORES          # 4 batches per core
CELLS = H * W                  # 4096 cells per batch
ROWLEN = 3 * (5 + NC_CLS)      # 255 floats per cell (3 anchor rows x 85)
P2 = 2 * M                     # 100 partitions: 2 batches x 50 boxes
NCONST = ROWLEN + P2 + P2 + 2  # packed const columns: cidx | ut2 | ident | boff
FP32 = mybir.dt.float32
I32 = mybir.dt.int32
Alu = mybir.AluOpType
Act = mybir.ActivationFunctionType

MAGIC = 8388608.0  # 2^23


def _host_consts():
    pk = np.zeros((P2, NCONST), dtype=np.float32)
    # cidx[*, a*85 + k] = k-5 for k in [5,85), else -1 (never matches a class)
    pk[:, 0:ROWLEN] = -1.0
    for a in range(3):
        pk[:, a * 85 + 5 : (a + 1) * 85] = np.arange(NC_CLS, dtype=np.float32)
    # ut2[p, q] = 1 iff same 50-block and q%50 > p%50 (strictly-later box)
    blk = np.arange(P2) // M
    mi = np.arange(P2) % M
    pk[:, ROWLEN : ROWLEN + P2] = (
        (blk[:, None] == blk[None, :]) & (mi[None, :] > mi[:, None])
    ).astype(np.float32)
    pk[:, ROWLEN + P2 : ROWLEN + 2 * P2] = np.eye(P2, dtype=np.float32)
    # cell offset per partition, per pair-slot: batch = 2*j + p//50
    for j in range(2):
        pk[:M, ROWLEN + 2 * P2 + j] = (2 * j) * CELLS
        pk[M:, ROWLEN + 2 * P2 + j] = (2 * j + 1) * CELLS
    return {"constpk": pk}


def _build_kernel_body(tc, x_ap, t_ap, out_ap, cpk_ap):
    nc = tc.nc
    from contextlib import ExitStack

    ctx = ExitStack()
    with ctx:
        consts = ctx.enter_context(tc.tile_pool(name="consts", bufs=1))
        work = ctx.enter_context(tc.tile_pool(name="work", bufs=2))
        gpool = ctx.enter_context(tc.tile_pool(name="gather", bufs=1))
        psum = ctx.enter_context(tc.tile_pool(name="psum", bufs=2, space="PSUM"))

        # ---- input DMAs: targets first (on Sync), consts in parallel (GpSimd)
        # targets: [100, 2, 5]; partition p = (batch-in-pair p//50, box p%50)
        tgt_t = consts.tile([P2, 2 * 5], FP32)
        nc.sync.dma_start(
            tgt_t[:].rearrange("p (j f) -> p j f", f=5),
            t_ap.rearrange("(j bb) m f -> (bb m) j f", j=2),
        )
        cpk_t = consts.tile([P2, NCONST], FP32)
        nc.gpsimd.dma_start(cpk_t[:], cpk_ap[:])
        cidx = cpk_t[:, 0:ROWLEN]
        ut = cpk_t[:, ROWLEN : ROWLEN + P2]
        ident = cpk_t[:, ROWLEN + P2 : ROWLEN + 2 * P2]
        boff = cpk_t[:, ROWLEN + 2 * P2 : ROWLEN + 2 * P2 + 2]

        tv = tgt_t[:].rearrange("p (j f) -> p j f", f=5)
        txy = tv[:, :, 1:3]  # [100, 2, 2] (x, y per pair-slot)

        # ---- box math on [100, 4] = (j, xy): exact branchless floor of t*64
        # via ri = RNE(v) (magic add/sub), floor = ri - (ri > v)
        v4 = work.tile([P2, 4], FP32, tag="v4")
        v4v = v4[:].rearrange("p (j c) -> p j c", c=2)
        nc.vector.tensor_scalar(v4v, txy, 64.0, None, op0=Alu.mult)
        ri4 = work.tile([P2, 4], FP32, tag="ri4")
        ri4v = ri4[:].rearrange("p (j c) -> p j c", c=2)
        nc.vector.tensor_scalar(ri4v, txy, 64.0, MAGIC, op0=Alu.mult, op1=Alu.add)
        nc.vector.tensor_scalar(ri4[:], ri4[:], MAGIC, None, op0=Alu.subtract)
        corr4 = work.tile([P2, 4], FP32, tag="corr4")
        nc.vector.tensor_tensor(corr4[:], ri4[:], v4[:], op=Alu.is_gt)
        fl4 = work.tile([P2, 4], FP32, tag="fl4")
        nc.vector.tensor_tensor(fl4[:], ri4[:], corr4[:], op=Alu.subtract)
        flv = fl4[:].rearrange("p (j c) -> p j c", c=2)  # [100, 2, 2] = (j, xy)

        # cell = y*64 + x (float), then celli = cell + batch_offset (int32)
        cellf = work.tile([P2, 2], FP32, tag="cellf")
        cfv = cellf[:].rearrange("p (j c) -> p j c", c=1)
        nc.vector.scalar_tensor_tensor(
            cfv, flv[:, :, 1:2], 64.0, flv[:, :, 0:1], op0=Alu.mult, op1=Alu.add
        )
        celli = work.tile([P2, 2], I32, tag="celli")
        nc.vector.tensor_tensor(celli[:], cellf[:], boff, op=Alu.add)

        # ---- gather both pair-slots' cell blocks ASAP (GpSimd queue is
        # empty after the const DMA) ----
        graw = gpool.tile([P2, 2 * ROWLEN], FP32, tag="graw")
        for j in range(2):
            nc.gpsimd.indirect_dma_start(
                out=graw[:, j * ROWLEN : (j + 1) * ROWLEN],
                out_offset=None,
                in_=x_ap,
                in_offset=bass.IndirectOffsetOnAxis(ap=celli[:, j : j + 1], axis=0),
            )

        # ---- winner resolution (last valid write wins) ----
        # valid[p,j] = sum(|t|) > 0 ; key = valid ? cell : -1
        val1 = work.tile([P2, 2], FP32, tag="val1")
        nc.vector.tensor_reduce(
            val1[:], tv, axis=mybir.AxisListType.X, op=Alu.add,
            apply_absolute_value=True,
        )
        valid = work.tile([P2, 2], FP32, tag="valid")
        nc.vector.tensor_scalar(valid[:], val1[:], 0.0, None, op0=Alu.is_gt)
        key = work.tile([P2, 2], FP32, tag="key")
        nc.vector.scalar_tensor_tensor(
            key[:], cellf[:], 1.0, valid[:], op0=Alu.add, op1=Alu.mult
        )
        nc.vector.tensor_scalar(key[:], key[:], -1.0, None, op0=Alu.add)

        # stats layout: [100, 10] = se(j,a) x 6 | winner x 2 | g3 x 2
        stats = consts.tile([P2, 10], FP32)

        qT0 = psum.tile([P2, P2], FP32, tag="qT0", space="PSUM")
        qT1 = psum.tile([P2, P2], FP32, tag="qT1", space="PSUM")
        qT = [qT0, qT1]
        for j in range(2):
            nc.tensor.transpose(
                qT[j][:], key[:, j : j + 1].to_broadcast([P2, P2]), ident
            )
        coll = work.tile([P2, 2], FP32, tag="coll")
        scrapV = work.tile([P2, ROWLEN], FP32, tag="scrapV")
        for j in range(2):
            # coll[p] = sum_q (key[q] == key[p]) * ut[p, q]  (later same-cell box)
            nc.vector.scalar_tensor_tensor(
                scrapV[:, 0:P2], qT[j][:], key[:, j : j + 1], ut,
                op0=Alu.is_equal, op1=Alu.mult, accum_out=coll[:, j : j + 1],
            )
        nc.vector.scalar_tensor_tensor(
            stats[:, 6:8], coll[:], 0.0, valid[:], op0=Alu.is_equal, op1=Alu.mult
        )

        # ---- per-(box, anchor) softmax denominators: se = sum_k exp(logit_k)
        scrapS = consts.tile([P2, NC_CLS], FP32)
        for j in range(2):
            for a in range(3):
                base = j * ROWLEN + a * 85 + 5
                nc.scalar.activation(
                    scrapS[:], graw[:, base : base + NC_CLS], Act.Exp,
                    accum_out=stats[:, 3 * j + a : 3 * j + a + 1],
                )

        # ---- label-logit sums: g3 = sum_k (cidx == cls) * graw ----
        for j in range(2):
            nc.vector.scalar_tensor_tensor(
                scrapV[:], cidx, tv[:, j, 0:1], graw[:, j * ROWLEN : (j + 1) * ROWLEN],
                op0=Alu.is_equal, op1=Alu.mult, accum_out=stats[:, 8 + j : 9 + j],
            )

        nc.scalar.dma_start(out_ap[:], stats[:])


_CACHE = {}


def _get_compiled():
    if "nc" in _CACHE:
        return _CACHE["nc"]
    nc = bacc.Bacc(
        "TRN2",
        target_bir_lowering=False,
        debug=False,
        enable_asserts=False,
        num_devices=N_CORES,
    )
    x = nc.dram_tensor("xflat", [B_CORE * CELLS, ROWLEN], FP32, kind="ExternalInput")
    t = nc.dram_tensor("tgt", [B_CORE, M, 5], FP32, kind="ExternalInput")
    cpk = nc.dram_tensor("constpk", [P2, NCONST], FP32, kind="ExternalInput")
    out = nc.dram_tensor("statsout", [P2, 10], FP32, kind="ExternalOutput")

    with tile.TileContext(nc) as tc:
        _build_kernel_body(tc, x.ap(), t.ap(), out.ap(), cpk.ap())
    nc.compile()
    _CACHE["nc"] = nc
    return nc


def _finish(stats_list):
    """Host: d = sum_a ln(se) - g3, per-batch mean, global mean (float64)."""
    total = 0.0
    for st in stats_list:
        st = np.asarray(st, dtype=np.float64)  # [100, 10]
        se = st[:, 0:6].reshape(P2, 2, 3)
        win = st[:, 6:8]
        g3 = st[:, 8:10]
        num = (np.log(np.maximum(se, 1e-300)).sum(axis=2) - g3) * win
        for j in range(2):
            for bb in range(2):
                sl = slice(bb * M, (bb + 1) * M)
                n = num[sl, j].sum()
                c = win[sl, j].sum()
                total += n / max(3.0 * c, 1.0)
    return total / B


def _run(output, targets, trace=False):
    nc = _get_compiled()
    consts = _host_consts()
    output = np.ascontiguousarray(output, dtype=np.float32)
    targets = np.ascontiguousarray(targets, dtype=np.float32)
    in_maps = []
    for k in range(N_CORES):
        in_maps.append(
            {
                "xflat": output[k * B_CORE : (k + 1) * B_CORE].reshape(
                    B_CORE * CELLS, ROWLEN
                ),
                "tgt": targets[k * B_CORE : (k + 1) * B_CORE],
                **consts,
            }
        )
    res = run_bass_kernel_spmd(nc, in_maps, core_ids=list(range(N_CORES)), trace=trace)
    total = _finish([r["statsout"] for r in res.results])
    return np.float32(total), res


def kernel(output, targets):
    val, _ = _run(output, targets)
    return np.asarray(val, dtype=np.float32)


# revision 10
# speedup vs baseline: 1.6169x; 1.0619x over previous
"""Trainium2 Bass kernel for nn_ClassLoss (YOLO-style classification CE loss).

Strategy: the loss depends only on grid cells hit by valid target boxes
(<=50 cells/batch out of 4096). Each cell corresponds to 3 consecutive
"flat rows" of the [12288, 85] logits block (765 contiguous floats in DRAM).
So instead of streaming 127MB of logits, each core:
  1. loads its 4 batches' targets as [100, 10] (partition = 2-batch-pair x
     50 boxes, free = pair-slot j x 5 fields) plus one packed const tensor,
  2. computes per-box (cell, class, valid) for both pair-slots at once on
     [100, 2] tiles (floor via fp32 mod), batch offsets via memset,
  3. indirect-DMA-gathers the two [100, 255] cell blocks (parallel SWDGE
     queues) into one tile,
  4. resolves last-write-wins duplicates with a PE transpose + fused
     (is_equal * upper-tri, accum) scalar_tensor_tensor per pair-slot,
  5. computes per-(box, anchor) softmax denominators with 2 Exp activations
     + X-axis reduces, and the label-logit sums with fused
     (cidx == cls) * graw accumulating stts,
  6. DMAs the per-box stats [100, 10] = (se x 6, winner x 2, g3 x 2).
Host finishes in float64: d = sum_a ln(se) - g3, per-batch mean
num / max(3*cnt, 1), sum over batches / B (the data-parallel all-reduce).
"""

import sys

sys.path.insert(0, "/opt/trn_rl_repo")

import numpy as np

import concourse.bass as bass
import concourse.tile as tile
from concourse import bacc, mybir
from concourse.bass_utils import run_bass_kernel_spmd

# Problem constants (hardcoded per harness contract).
B, A, H, W, NC_CLS, M = 32, 3, 64, 64, 80, 50
N_CORES = 8
B_CORE = B // N_CORES          # 4 batches per core
CELLS = H * W                  # 4096 cells per batch
ROWLEN = 3 * (5 + NC_CLS)      # 255 floats per cell (3 anchor rows x 85)
P2 = 2 * M                     # 100 partitions: 2 batches x 50 boxes
NCONST = ROWLEN + P2 + P2      # packed const columns: cidx | ut2 | ident
FP32 = mybir.dt.float32
I32 = mybir.dt.int32
Alu = mybir.AluOpType
Act = mybir.ActivationFunctionType

MAGIC = 8388608.0  # 2^23


def _host_consts():
    pk = np.zeros((P2, NCONST), dtype=np.float32)
    # cidx[*, a*85 + k] = k-5 for k in [5,85), else -1 (never matches a class)
    pk[:, 0:ROWLEN] = -1.0
    for a in range(3):
        pk[:, a * 85 + 5 : (a + 1) * 85] = np.arange(NC_CLS, dtype=np.float32)
    # ut2[p, q] = 1 iff same 50-block and q%50 > p%50 (strictly-later box)
    blk = np.arange(P2) // M
    mi = np.arange(P2) % M
    pk[:, ROWLEN : ROWLEN + P2] = (
        (blk[:, None] == blk[None, :]) & (mi[None, :] > mi[:, None])
    ).astype(np.float32)
    pk[:, ROWLEN + P2 : ROWLEN + 2 * P2] = np.eye(P2, dtype=np.float32)
    boff = np.empty((P2, 2), dtype=np.float32)
    for j in range(2):
        boff[:M, j] = (2 * j) * CELLS
        boff[M:, j] = (2 * j + 1) * CELLS
    return {"constpk": pk, "boff": boff}


def _build_kernel_body(tc, x_ap, t_ap, out_ap, cpk_ap, boff_ap):
    nc = tc.nc
    from contextlib import ExitStack

    ctx = ExitStack()
    with ctx:
        consts = ctx.enter_context(tc.tile_pool(name="consts", bufs=1))
        work = ctx.enter_context(tc.tile_pool(name="work", bufs=2))
        gpool = ctx.enter_context(tc.tile_pool(name="gather", bufs=1))
        psum = ctx.enter_context(tc.tile_pool(name="psum", bufs=2, space="PSUM"))

        # ---- input DMAs: targets first (on Sync), consts in parallel (GpSimd)
        # targets: [100, 2, 5]; partition p = (batch-in-pair p//50, box p%50)
        tgt_t = consts.tile([P2, 2 * 5], FP32)
        nc.sync.dma_start(
            tgt_t[:].rearrange("p (j f) -> p j f", f=5),
            t_ap.rearrange("(j bb) m f -> (bb m) j f", j=2),
        )
        cpk_t = consts.tile([P2, NCONST], FP32)
        nc.gpsimd.dma_start(cpk_t[:], cpk_ap[:])
        cidx = cpk_t[:, 0:ROWLEN]
        ut = cpk_t[:, ROWLEN : ROWLEN + P2]
        ident = cpk_t[:, ROWLEN + P2 : ROWLEN + 2 * P2]

        # batch cell-offsets (boff[p, j] = (2j + p//50) * 4096) via a small
        # separate DMA (2nd on Sync) so celli doesn't wait on the big const DMA
        boff = consts.tile([P2, 2], FP32)
        nc.sync.dma_start(boff[:], boff_ap[:])

        tv = tgt_t[:].rearrange("p (j f) -> p j f", f=5)
        txy = tv[:, :, 1:3]  # [100, 2, 2] (x, y per pair-slot)

        # ---- box math on [100, 4] = (j, xy): exact branchless floor of t*64
        # via ri = RNE(v) (magic add/sub), floor = ri - (ri > v)
        v4 = work.tile([P2, 4], FP32, tag="v4")
        v4v = v4[:].rearrange("p (j c) -> p j c", c=2)
        nc.vector.tensor_scalar(v4v, txy, 64.0, None, op0=Alu.mult)
        ri4 = work.tile([P2, 4], FP32, tag="ri4")
        ri4v = ri4[:].rearrange("p (j c) -> p j c", c=2)
        nc.vector.tensor_scalar(ri4v, txy, 64.0, MAGIC, op0=Alu.mult, op1=Alu.add)
        nc.vector.tensor_scalar(ri4[:], ri4[:], MAGIC, None, op0=Alu.subtract)
        corr4 = work.tile([P2, 4], FP32, tag="corr4")
        nc.vector.tensor_tensor(corr4[:], ri4[:], v4[:], op=Alu.is_gt)
        fl4 = work.tile([P2, 4], FP32, tag="fl4")
        nc.vector.tensor_tensor(fl4[:], ri4[:], corr4[:], op=Alu.subtract)
        flv = fl4[:].rearrange("p (j c) -> p j c", c=2)  # [100, 2, 2] = (j, xy)

        # cell = y*64 + x (float), then celli = cell + batch_offset (int32)
        cellf = work.tile([P2, 2], FP32, tag="cellf")
        cfv = cellf[:].rearrange("p (j c) -> p j c", c=1)
        nc.vector.scalar_tensor_tensor(
            cfv, flv[:, :, 1:2], 64.0, flv[:, :, 0:1], op0=Alu.mult, op1=Alu.add
        )
        celli = work.tile([P2, 2], I32, tag="celli")
        nc.vector.tensor_tensor(celli[:], cellf[:], boff[:], op=Alu.add)

        # ---- gather both pair-slots' cell blocks ASAP (GpSimd queue is
        # empty after the const DMA); second gather on a parallel SWDGE queue
        graw = gpool.tile([P2, 2 * ROWLEN], FP32, tag="graw")
        for j in range(2):
            inst = nc.gpsimd.indirect_dma_start(
                out=graw[:, j * ROWLEN : (j + 1) * ROWLEN],
                out_offset=None,
                in_=x_ap,
                in_offset=bass.IndirectOffsetOnAxis(ap=celli[:, j : j + 1], axis=0),
            )

        # ---- winner resolution (last valid write wins) ----
        # valid[p,j] = sum(|t|) > 0 ; key = valid ? cell : -1
        val1 = work.tile([P2, 2], FP32, tag="val1")
        nc.vector.tensor_reduce(
            val1[:], tv, axis=mybir.AxisListType.X, op=Alu.add,
            apply_absolute_value=True,
        )
        valid = work.tile([P2, 2], FP32, tag="valid")
        nc.vector.tensor_scalar(valid[:], val1[:], 0.0, None, op0=Alu.is_gt)
        key = work.tile([P2, 2], FP32, tag="key")
        nc.vector.scalar_tensor_tensor(
            key[:], cellf[:], 1.0, valid[:], op0=Alu.add, op1=Alu.mult
        )
        nc.vector.tensor_scalar(key[:], key[:], -1.0, None, op0=Alu.add)

        # stats layout: [100, 10] = se(j,a) x 6 | winner x 2 | g3 x 2
        stats = consts.tile([P2, 10], FP32)

        qT0 = psum.tile([P2, P2], FP32, tag="qT0", space="PSUM")
        qT1 = psum.tile([P2, P2], FP32, tag="qT1", space="PSUM")
        qT = [qT0, qT1]
        for j in range(2):
            nc.tensor.transpose(
                qT[j][:], key[:, j : j + 1].to_broadcast([P2, P2]), ident
            )
        coll = work.tile([P2, 2], FP32, tag="coll")
        scrapV = work.tile([P2, ROWLEN], FP32, tag="scrapV")
        for j in range(2):
            # coll[p] = sum_q (key[q] == key[p]) * ut[p, q]  (later same-cell box)
            nc.vector.scalar_tensor_tensor(
                scrapV[:, 0:P2], qT[j][:], key[:, j : j + 1], ut,
                op0=Alu.is_equal, op1=Alu.mult, accum_out=coll[:, j : j + 1],
            )
        nc.vector.scalar_tensor_tensor(
            stats[:, 6:8], coll[:], 0.0, valid[:], op0=Alu.is_equal, op1=Alu.mult
        )

        # ---- per-(box, anchor) softmax denominators: se = sum_k exp(logit_k)
        # 2 big Exp activations (one per pair-slot) + X-axis reduces, with the
        # label-logit stts interleaved by data arrival
        ex = gpool.tile([P2, 2 * 3 * NC_CLS], FP32, tag="ex")
        for j in range(2):
            gv = graw[:, j * ROWLEN : (j + 1) * ROWLEN].rearrange(
                "p (a f) -> p a f", f=85
            )[:, :, 5:]
            exv = ex[:, j * 3 * NC_CLS : (j + 1) * 3 * NC_CLS].rearrange(
                "p (a f) -> p a f", f=NC_CLS
            )
            nc.scalar.activation(exv, gv, Act.Exp)
            # g3 = sum_k (cidx == cls) * graw  (label-logit sum over 3 anchors)
            nc.vector.scalar_tensor_tensor(
                scrapV[:], cidx, tv[:, j, 0:1],
                graw[:, j * ROWLEN : (j + 1) * ROWLEN],
                op0=Alu.is_equal, op1=Alu.mult, accum_out=stats[:, 8 + j : 9 + j],
            )
            nc.vector.tensor_reduce(
                stats[:, 3 * j : 3 * j + 3], exv, axis=mybir.AxisListType.X,
                op=Alu.add,
            )

        nc.sync.dma_start(out_ap[:], stats[:])


_CACHE = {}


def _get_compiled():
    if "nc" in _CACHE:
        return _CACHE["nc"]
    nc = bacc.Bacc(
        "TRN2",
        target_bir_lowering=False,
        debug=False,
        enable_asserts=False,
        num_devices=N_CORES,
    )
    x = nc.dram_tensor("xflat", [B_CORE * CELLS, ROWLEN], FP32, kind="ExternalInput")
    t = nc.dram_tensor("tgt", [B_CORE, M, 5], FP32, kind="ExternalInput")
    cpk = nc.dram_tensor("constpk", [P2, NCONST], FP32, kind="ExternalInput")
    boff = nc.dram_tensor("boff", [P2, 2], FP32, kind="ExternalInput")
    out = nc.dram_tensor("statsout", [P2, 10], FP32, kind="ExternalOutput")

    with tile.TileContext(nc) as tc:
        _build_kernel_body(tc, x.ap(), t.ap(), out.ap(), cpk.ap(), boff.ap())
    nc.compile()
    _CACHE["nc"] = nc
    return nc


def _finish(stats_list):
    """Host: d = sum_a ln(se) - g3, per-batch mean, global mean (float64)."""
    total = 0.0
    for st in stats_list:
        st = np.asarray(st, dtype=np.float64)  # [100, 10]
        se = st[:, 0:6].reshape(P2, 2, 3)
        win = st[:, 6:8]
        g3 = st[:, 8:10]
        num = (np.log(np.maximum(se, 1e-300)).sum(axis=2) - g3) * win
        for j in range(2):
            for bb in range(2):
                sl = slice(bb * M, (bb + 1) * M)
                n = num[sl, j].sum()
                c = win[sl, j].sum()
                total += n / max(3.0 * c, 1.0)
    return total / B


def _run(output, targets, trace=False):
    nc = _get_compiled()
    consts = _host_consts()
    output = np.ascontiguousarray(output, dtype=np.float32)
    targets = np.ascontiguousarray(targets, dtype=np.float32)
    in_maps = []
    for k in range(N_CORES):
        in_maps.append(
            {
                "xflat": output[k * B_CORE : (k + 1) * B_CORE].reshape(
                    B_CORE * CELLS, ROWLEN
                ),
                "tgt": targets[k * B_CORE : (k + 1) * B_CORE],
                **consts,
            }
        )
    res = run_bass_kernel_spmd(nc, in_maps, core_ids=list(range(N_CORES)), trace=trace)
    total = _finish([r["statsout"] for r in res.results])
    return np.float32(total), res


def kernel(output, targets):
    val, _ = _run(output, targets)
    return np.asarray(val, dtype=np.float32)


# revision 12
# speedup vs baseline: 1.6732x; 1.0348x over previous
"""Trainium2 Bass kernel for nn_ClassLoss (YOLO-style classification CE loss).

Strategy: the loss depends only on grid cells hit by valid target boxes
(<=50 cells/batch out of 4096). Each cell corresponds to 3 consecutive
"flat rows" of the [12288, 85] logits block (765 contiguous floats in DRAM).
So instead of streaming 127MB of logits, each core:
  1. loads its 4 batches' targets as [100, 10] (partition = 2-batch-pair x
     50 boxes, free = pair-slot j x 5 fields) plus one packed const tensor,
  2. computes per-box (cell, class, valid) for both pair-slots at once on
     [100, 2] tiles (floor via fp32 mod), batch offsets via iota,
  3. indirect-DMA-gathers the two [100, 255] cell blocks into one tile,
     with invalid boxes pointed out-of-bounds so the bounds check skips
     their DRAM reads,
  4. resolves last-write-wins duplicates with a PE transpose + fused
     (is_equal * upper-tri, accum) scalar_tensor_tensor per pair-slot,
  5. computes per-(box, anchor) softmax denominators with 2 Exp activations
     + X-axis reduces, and the label-logit sums with fused
     (cidx == cls) * graw accumulating stts,
  6. DMAs the per-box stats [100, 10] = (se x 6, winner x 2, g3 x 2).
Host finishes in float64: d = sum_a ln(se) - g3, per-batch mean
num / max(3*cnt, 1), sum over batches / B (the data-parallel all-reduce).
"""

import sys

sys.path.insert(0, "/opt/trn_rl_repo")

import numpy as np

import concourse.bass as bass
import concourse.tile as tile
from concourse import bacc, mybir
from concourse.bass_utils import run_bass_kernel_spmd

# Problem constants (hardcoded per harness contract).
B, A, H, W, NC_CLS, M = 32, 3, 64, 64, 80, 50
N_CORES = 8
B_CORE = B // N_CORES          # 4 batches per core
CELLS = H * W                  # 4096 cells per batch
ROWLEN = 3 * (5 + NC_CLS)      # 255 floats per cell (3 anchor rows x 85)
P2 = 2 * M                     # 100 partitions: 2 batches x 50 boxes
NCONST = ROWLEN + P2 + P2      # packed const columns: cidx | ut2 | ident
FP32 = mybir.dt.float32
I32 = mybir.dt.int32
Alu = mybir.AluOpType
Act = mybir.ActivationFunctionType



def _host_consts():
    pk = np.zeros((P2, NCONST), dtype=np.float32)
    # cidx[*, a*85 + k] = k-5 for k in [5,85), else -1 (never matches a class)
    pk[:, 0:ROWLEN] = -1.0
    for a in range(3):
        pk[:, a * 85 + 5 : (a + 1) * 85] = np.arange(NC_CLS, dtype=np.float32)
    # ut2[p, q] = 1 iff same 50-block and q%50 > p%50 (strictly-later box)
    blk = np.arange(P2) // M
    mi = np.arange(P2) % M
    pk[:, ROWLEN : ROWLEN + P2] = (
        (blk[:, None] == blk[None, :]) & (mi[None, :] > mi[:, None])
    ).astype(np.float32)
    pk[:, ROWLEN + P2 : ROWLEN + 2 * P2] = np.eye(P2, dtype=np.float32)
    return {"constpk": pk}


def _build_kernel_body(tc, x_ap, t_ap, out_ap, cpk_ap):
    nc = tc.nc
    from contextlib import ExitStack

    ctx = ExitStack()
    with ctx:
        consts = ctx.enter_context(tc.tile_pool(name="consts", bufs=1))
        work = ctx.enter_context(tc.tile_pool(name="work", bufs=2))
        gpool = ctx.enter_context(tc.tile_pool(name="gather", bufs=1))
        psum = ctx.enter_context(tc.tile_pool(name="psum", bufs=2, space="PSUM"))

        # ---- input DMAs: targets first (on Sync), consts in parallel (GpSimd)
        # targets: [100, 2, 5]; partition p = (batch-in-pair p//50, box p%50)
        tgt_t = consts.tile([P2, 2 * 5], FP32)
        nc.sync.dma_start(
            tgt_t[:].rearrange("p (j f) -> p j f", f=5),
            t_ap.rearrange("(j bb) m f -> (bb m) j f", j=2),
        )
        cpk_t = consts.tile([P2, NCONST], FP32)
        nc.gpsimd.dma_start(cpk_t[:], cpk_ap[:])
        cidx = cpk_t[:, 0:ROWLEN]
        ut = cpk_t[:, ROWLEN : ROWLEN + P2]
        ident = cpk_t[:, ROWLEN + P2 : ROWLEN + 2 * P2]

        # batch cell-offsets (boff[p, j] = (2j + p//50) * 4096) generated
        # on-chip (iota partition index + 3 tiny vector ops) so celli never
        # waits on a DMA
        pidx = consts.tile([P2, 1], I32)
        nc.gpsimd.iota(pidx[:], [[0, 1]], base=0, channel_multiplier=1)
        bh = consts.tile([P2, 1], FP32)
        nc.vector.tensor_scalar(bh[:], pidx[:], float(M - 1), None, op0=Alu.is_gt)
        boff = consts.tile([P2, 2], FP32)
        nc.vector.tensor_scalar(boff[:, 0:1], bh[:], float(CELLS), None, op0=Alu.mult)
        nc.vector.tensor_scalar(
            boff[:, 1:2], bh[:], float(CELLS), float(2 * CELLS),
            op0=Alu.mult, op1=Alu.add,
        )

        tv = tgt_t[:].rearrange("p (j f) -> p j f", f=5)
        txy = tv[:, :, 1:3]  # [100, 2, 2] (x, y per pair-slot)

        # ---- box math on [100, 4] = (j, xy): exact branchless floor of t*64
        # via ri = RNE(v) (magic add/sub), floor = ri - (ri > v)
        MAGIC = 8388608.0  # 2^23
        v4 = work.tile([P2, 4], FP32, tag="v4")
        v4v = v4[:].rearrange("p (j c) -> p j c", c=2)
        nc.vector.tensor_scalar(v4v, txy, 64.0, None, op0=Alu.mult)
        ri4 = work.tile([P2, 4], FP32, tag="ri4")
        ri4v = ri4[:].rearrange("p (j c) -> p j c", c=2)
        nc.vector.tensor_scalar(ri4v, txy, 64.0, MAGIC, op0=Alu.mult, op1=Alu.add)
        nc.vector.tensor_scalar(ri4[:], ri4[:], MAGIC, None, op0=Alu.subtract)
        corr4 = work.tile([P2, 4], FP32, tag="corr4")
        nc.vector.tensor_tensor(corr4[:], ri4[:], v4[:], op=Alu.is_gt)
        fl4 = work.tile([P2, 4], FP32, tag="fl4")
        nc.vector.tensor_tensor(fl4[:], ri4[:], corr4[:], op=Alu.subtract)
        flv = fl4[:].rearrange("p (j c) -> p j c", c=2)  # [100, 2, 2] = (j, xy)

        # cell = y*64 + x (float), then celli = cell + batch_offset (int32)
        cellf = work.tile([P2, 2], FP32, tag="cellf")
        cfv = cellf[:].rearrange("p (j c) -> p j c", c=1)
        nc.vector.scalar_tensor_tensor(
            cfv, flv[:, :, 1:2], 64.0, flv[:, :, 0:1], op0=Alu.mult, op1=Alu.add
        )
        # valid[p,j] = sum(|t|) > 0; invalid boxes get an out-of-bounds cell
        # index so the gather's bounds check skips their DRAM reads entirely
        val1 = work.tile([P2, 2], FP32, tag="val1")
        nc.vector.tensor_reduce(
            val1[:], tv, axis=mybir.AxisListType.X, op=Alu.add,
            apply_absolute_value=True,
        )
        valid = work.tile([P2, 2], FP32, tag="valid")
        nc.vector.tensor_scalar(valid[:], val1[:], 0.0, None, op0=Alu.is_gt)
        oobm = work.tile([P2, 2], FP32, tag="oobm")
        nc.vector.tensor_scalar(
            oobm[:], valid[:], -1048576.0, 1048576.0, op0=Alu.mult, op1=Alu.add
        )
        celli0 = work.tile([P2, 2], FP32, tag="celli0")
        nc.vector.tensor_tensor(celli0[:], cellf[:], boff[:], op=Alu.add)
        celli = work.tile([P2, 2], I32, tag="celli")
        nc.vector.tensor_tensor(celli[:], celli0[:], oobm[:], op=Alu.add)

        # ---- gather both pair-slots' cell blocks ASAP (GpSimd queue is
        # empty after the const DMA); second gather on a parallel SWDGE queue
        graw = gpool.tile([P2, 2 * ROWLEN], FP32, tag="graw")
        for j in range(2):
            nc.gpsimd.indirect_dma_start(
                out=graw[:, j * ROWLEN : (j + 1) * ROWLEN],
                out_offset=None,
                in_=x_ap,
                in_offset=bass.IndirectOffsetOnAxis(ap=celli[:, j : j + 1], axis=0),
                bounds_check=B_CORE * CELLS - 1,
                oob_is_err=False,
            )

        # ---- winner resolution (last valid write wins) ----
        # key = valid ? cell : -1
        key = work.tile([P2, 2], FP32, tag="key")
        nc.vector.scalar_tensor_tensor(
            key[:], cellf[:], 1.0, valid[:], op0=Alu.add, op1=Alu.mult
        )
        nc.vector.tensor_scalar(key[:], key[:], -1.0, None, op0=Alu.add)

        # stats layout: [100, 10] = se(j,a) x 6 | winner x 2 | g3 x 2
        stats = consts.tile([P2, 10], FP32)

        qT0 = psum.tile([P2, P2], FP32, tag="qT0", space="PSUM")
        qT1 = psum.tile([P2, P2], FP32, tag="qT1", space="PSUM")
        qT = [qT0, qT1]
        for j in range(2):
            nc.tensor.transpose(
                qT[j][:], key[:, j : j + 1].to_broadcast([P2, P2]), ident
            )
        coll = work.tile([P2, 2], FP32, tag="coll")
        scrapV = work.tile([P2, ROWLEN], FP32, tag="scrapV")
        for j in range(2):
            # coll[p] = sum_q (key[q] == key[p]) * ut[p, q]  (later same-cell box)
            nc.vector.scalar_tensor_tensor(
                scrapV[:, 0:P2], qT[j][:], key[:, j : j + 1], ut,
                op0=Alu.is_equal, op1=Alu.mult, accum_out=coll[:, j : j + 1],
            )
        nc.vector.scalar_tensor_tensor(
            stats[:, 6:8], coll[:], 0.0, valid[:], op0=Alu.is_equal, op1=Alu.mult
        )

        # ---- per-(box, anchor) softmax denominators: se = sum_k exp(logit_k)
        # 2 big Exp activations (one per pair-slot) + X-axis reduces, with the
        # label-logit stts interleaved by data arrival
        ex = gpool.tile([P2, 2 * 3 * NC_CLS], FP32, tag="ex")
        for j in range(2):
            gv = graw[:, j * ROWLEN : (j + 1) * ROWLEN].rearrange(
                "p (a f) -> p a f", f=85
            )[:, :, 5:]
            exv = ex[:, j * 3 * NC_CLS : (j + 1) * 3 * NC_CLS].rearrange(
                "p (a f) -> p a f", f=NC_CLS
            )
            nc.scalar.activation(exv, gv, Act.Exp)
            # g3 = sum_k (cidx == cls) * graw  (label-logit sum over 3 anchors)
            nc.vector.scalar_tensor_tensor(
                scrapV[:], cidx, tv[:, j, 0:1],
                graw[:, j * ROWLEN : (j + 1) * ROWLEN],
                op0=Alu.is_equal, op1=Alu.mult, accum_out=stats[:, 8 + j : 9 + j],
            )
            nc.vector.tensor_reduce(
                stats[:, 3 * j : 3 * j + 3], exv, axis=mybir.AxisListType.X,
                op=Alu.add,
            )

        nc.sync.dma_start(out_ap[:], stats[:])


_CACHE = {}


def _get_compiled():
    if "nc" in _CACHE:
        return _CACHE["nc"]
    nc = bacc.Bacc(
        "TRN2",
        target_bir_lowering=False,
        debug=False,
        enable_asserts=False,
        num_devices=N_CORES,
    )
    x = nc.dram_tensor("xflat", [B_CORE * CELLS, ROWLEN], FP32, kind="ExternalInput")
    t = nc.dram_tensor("tgt", [B_CORE, M, 5], FP32, kind="ExternalInput")
    cpk = nc.dram_tensor("constpk", [P2, NCONST], FP32, kind="ExternalInput")
    out = nc.dram_tensor("statsout", [P2, 10], FP32, kind="ExternalOutput")

    with tile.TileContext(nc) as tc:
        _build_kernel_body(tc, x.ap(), t.ap(), out.ap(), cpk.ap())
    nc.compile()
    _CACHE["nc"] = nc
    return nc


def _finish(stats_list):
    """Host: d = sum_a ln(se) - g3, per-batch mean, global mean (float64)."""
    total = 0.0
    for st in stats_list:
        st = np.asarray(st, dtype=np.float64)  # [100, 10]
        se = st[:, 0:6].reshape(P2, 2, 3)
        win = st[:, 6:8]
        g3 = st[:, 8:10]
        with np.errstate(all="ignore"):
            lnse = np.log(np.maximum(se, 1e-300)).sum(axis=2)
        num = np.where(win > 0.0, lnse - g3, 0.0)
        for j in range(2):
            for bb in range(2):
                sl = slice(bb * M, (bb + 1) * M)
                n = num[sl, j].sum()
                c = win[sl, j].sum()
                total += n / max(3.0 * c, 1.0)
    return total / B


def _run(output, targets, trace=False):
    nc = _get_compiled()
    consts = _host_consts()
    output = np.ascontiguousarray(output, dtype=np.float32)
    targets = np.ascontiguousarray(targets, dtype=np.float32)
    in_maps = []
    for k in range(N_CORES):
        in_maps.append(
            {
                "xflat": output[k * B_CORE : (k + 1) * B_CORE].reshape(
                    B_CORE * CELLS, ROWLEN
                ),
                "tgt": targets[k * B_CORE : (k + 1) * B_CORE],
                **consts,
            }
        )
    res = run_bass_kernel_spmd(nc, in_maps, core_ids=list(range(N_CORES)), trace=trace)
    total = _finish([r["statsout"] for r in res.results])
    return np.float32(total), res


def kernel(output, targets):
    val, _ = _run(output, targets)
    return np.asarray(val, dtype=np.float32)


# revision 13
# speedup vs baseline: 1.6773x; 1.0024x over previous
"""Trainium2 Bass kernel for nn_ClassLoss (YOLO-style classification CE loss).

Strategy: the loss depends only on grid cells hit by valid target boxes
(<=50 cells/batch out of 4096). Each cell corresponds to 3 consecutive
"flat rows" of the [12288, 85] logits block (765 contiguous floats in DRAM).
So instead of streaming 127MB of logits, each core:
  1. loads its 4 batches' targets as [100, 10] (partition = 2-batch-pair x
     50 boxes, free = pair-slot j x 5 fields) plus one packed const tensor,
  2. computes per-box (cell, class, valid) for both pair-slots at once on
     [100, 2] tiles (floor via fp32 mod), batch offsets via iota,
  3. indirect-DMA-gathers the two [100, 255] cell blocks into one tile,
     with invalid boxes pointed out-of-bounds so the bounds check skips
     their DRAM reads,
  4. resolves last-write-wins duplicates with a PE transpose + fused
     (is_equal * upper-tri, accum) scalar_tensor_tensor per pair-slot,
  5. computes per-(box, anchor) softmax denominators with 2 Exp activations
     + X-axis reduces, and the label-logit sums with fused
     (cidx == cls) * graw accumulating stts,
  6. DMAs the per-box stats [100, 10] = (se x 6, winner x 2, g3 x 2).
Host finishes in float64: d = sum_a ln(se) - g3, per-batch mean
num / max(3*cnt, 1), sum over batches / B (the data-parallel all-reduce).
"""

import sys

sys.path.insert(0, "/opt/trn_rl_repo")

import numpy as np

import concourse.bass as bass
import concourse.tile as tile
from concourse import bacc, mybir
from concourse.bass_utils import run_bass_kernel_spmd

# Problem constants (hardcoded per harness contract).
B, A, H, W, NC_CLS, M = 32, 3, 64, 64, 80, 50
N_CORES = 8
B_CORE = B // N_CORES          # 4 batches per core
CELLS = H * W                  # 4096 cells per batch
ROWLEN = 3 * (5 + NC_CLS)      # 255 floats per cell (3 anchor rows x 85)
P2 = 2 * M                     # 100 partitions: 2 batches x 50 boxes
NCONST = ROWLEN + P2 + P2      # packed const columns: cidx | ut2 | ident
FP32 = mybir.dt.float32
I32 = mybir.dt.int32
Alu = mybir.AluOpType
Act = mybir.ActivationFunctionType



def _host_consts():
    pk = np.zeros((P2, NCONST), dtype=np.float32)
    # cidx[*, a*85 + k] = k-5 for k in [5,85), else -1 (never matches a class)
    pk[:, 0:ROWLEN] = -1.0
    for a in range(3):
        pk[:, a * 85 + 5 : (a + 1) * 85] = np.arange(NC_CLS, dtype=np.float32)
    # ut2[p, q] = 1 iff same 50-block and q%50 > p%50 (strictly-later box)
    blk = np.arange(P2) // M
    mi = np.arange(P2) % M
    pk[:, ROWLEN : ROWLEN + P2] = (
        (blk[:, None] == blk[None, :]) & (mi[None, :] > mi[:, None])
    ).astype(np.float32)
    pk[:, ROWLEN + P2 : ROWLEN + 2 * P2] = np.eye(P2, dtype=np.float32)
    return {"constpk": pk}


def _build_kernel_body(tc, x_ap, t_ap, out_ap, cpk_ap):
    nc = tc.nc
    from contextlib import ExitStack

    ctx = ExitStack()
    with ctx:
        consts = ctx.enter_context(tc.tile_pool(name="consts", bufs=1))
        work = ctx.enter_context(tc.tile_pool(name="work", bufs=2))
        gpool = ctx.enter_context(tc.tile_pool(name="gather", bufs=1))
        psum = ctx.enter_context(tc.tile_pool(name="psum", bufs=2, space="PSUM"))

        # ---- input DMAs: targets first (on Sync), consts in parallel (GpSimd)
        # targets: [100, 2, 5]; partition p = (batch-in-pair p//50, box p%50)
        tgt_t = consts.tile([P2, 2 * 5], FP32)
        nc.sync.dma_start(
            tgt_t[:].rearrange("p (j f) -> p j f", f=5),
            t_ap.rearrange("(j bb) m f -> (bb m) j f", j=2),
        )
        cpk_t = consts.tile([P2, NCONST], FP32)
        nc.gpsimd.dma_start(cpk_t[:], cpk_ap[:])
        cidx = cpk_t[:, 0:ROWLEN]
        ut = cpk_t[:, ROWLEN : ROWLEN + P2]
        ident = cpk_t[:, ROWLEN + P2 : ROWLEN + 2 * P2]

        # batch cell-offsets (boff[p, j] = (2j + p//50) * 4096) generated
        # on-chip (iota partition index + 3 tiny vector ops) so celli never
        # waits on a DMA
        pidx = consts.tile([P2, 1], I32)
        nc.gpsimd.iota(pidx[:], [[0, 1]], base=0, channel_multiplier=1)
        bh = consts.tile([P2, 1], FP32)
        nc.vector.tensor_scalar(bh[:], pidx[:], float(M - 1), None, op0=Alu.is_gt)
        boff = consts.tile([P2, 2], FP32)
        nc.vector.tensor_scalar(boff[:, 0:1], bh[:], float(CELLS), None, op0=Alu.mult)
        nc.vector.tensor_scalar(
            boff[:, 1:2], bh[:], float(CELLS), float(2 * CELLS),
            op0=Alu.mult, op1=Alu.add,
        )

        tv = tgt_t[:].rearrange("p (j f) -> p j f", f=5)
        txy = tv[:, :, 1:3]  # [100, 2, 2] (x, y per pair-slot)

        # ---- box math on [100, 4] = (j, xy): exact branchless floor of t*64
        # via ri = RNE(v) (magic add/sub), floor = ri - (ri > v)
        MAGIC = 8388608.0  # 2^23
        v4 = work.tile([P2, 4], FP32, tag="v4")
        v4v = v4[:].rearrange("p (j c) -> p j c", c=2)
        nc.vector.tensor_scalar(v4v, txy, 64.0, None, op0=Alu.mult)
        ri4 = work.tile([P2, 4], FP32, tag="ri4")
        ri4v = ri4[:].rearrange("p (j c) -> p j c", c=2)
        nc.vector.tensor_scalar(ri4v, txy, 64.0, MAGIC, op0=Alu.mult, op1=Alu.add)
        nc.vector.tensor_scalar(ri4[:], ri4[:], MAGIC, None, op0=Alu.subtract)
        corr4 = work.tile([P2, 4], FP32, tag="corr4")
        nc.vector.tensor_tensor(corr4[:], ri4[:], v4[:], op=Alu.is_gt)
        fl4 = work.tile([P2, 4], FP32, tag="fl4")
        nc.vector.tensor_tensor(fl4[:], ri4[:], corr4[:], op=Alu.subtract)
        flv = fl4[:].rearrange("p (j c) -> p j c", c=2)  # [100, 2, 2] = (j, xy)

        # cell = y*64 + x (float), then celli = cell + batch_offset (int32)
        cellf = work.tile([P2, 2], FP32, tag="cellf")
        cfv = cellf[:].rearrange("p (j c) -> p j c", c=1)
        nc.vector.scalar_tensor_tensor(
            cfv, flv[:, :, 1:2], 64.0, flv[:, :, 0:1], op0=Alu.mult, op1=Alu.add
        )
        # valid[p,j] = sum(|t|) > 0; invalid boxes get an out-of-bounds cell
        # index so the gather's bounds check skips their DRAM reads entirely
        val1 = work.tile([P2, 2], FP32, tag="val1")
        nc.vector.tensor_reduce(
            val1[:], tv, axis=mybir.AxisListType.X, op=Alu.add,
            apply_absolute_value=True,
        )
        valid = work.tile([P2, 2], FP32, tag="valid")
        nc.vector.tensor_scalar(valid[:], val1[:], 0.0, None, op0=Alu.is_gt)
        oobm = work.tile([P2, 2], FP32, tag="oobm")
        nc.vector.tensor_scalar(
            oobm[:], valid[:], -1048576.0, 1048576.0, op0=Alu.mult, op1=Alu.add
        )
        bo2 = work.tile([P2, 2], FP32, tag="bo2")
        nc.vector.tensor_tensor(bo2[:], boff[:], oobm[:], op=Alu.add)
        celli = work.tile([P2, 2], I32, tag="celli")
        nc.vector.tensor_tensor(celli[:], cellf[:], bo2[:], op=Alu.add)

        # ---- gather both pair-slots' cell blocks ASAP (GpSimd queue is
        # empty after the const DMA); second gather on a parallel SWDGE queue
        graw = gpool.tile([P2, 2 * ROWLEN], FP32, tag="graw")
        for j in range(2):
            gi = nc.gpsimd.indirect_dma_start(
                out=graw[:, j * ROWLEN : (j + 1) * ROWLEN],
                out_offset=None,
                in_=x_ap,
                in_offset=bass.IndirectOffsetOnAxis(ap=celli[:, j : j + 1], axis=0),
                bounds_check=B_CORE * CELLS - 1,
                oob_is_err=False,
            )
            if j == 1:
                gi.ins.queue = "qPoolDynamic1"

        # ---- winner resolution (last valid write wins) ----
        # key = valid ? cell : -1
        key = work.tile([P2, 2], FP32, tag="key")
        nc.vector.scalar_tensor_tensor(
            key[:], cellf[:], 1.0, valid[:], op0=Alu.add, op1=Alu.mult
        )
        nc.vector.tensor_scalar(key[:], key[:], -1.0, None, op0=Alu.add)

        # stats layout: [100, 10] = se(j,a) x 6 | winner x 2 | g3 x 2
        stats = consts.tile([P2, 10], FP32)

        qT0 = psum.tile([P2, P2], FP32, tag="qT0", space="PSUM")
        qT1 = psum.tile([P2, P2], FP32, tag="qT1", space="PSUM")
        qT = [qT0, qT1]
        for j in range(2):
            nc.tensor.transpose(
                qT[j][:], key[:, j : j + 1].to_broadcast([P2, P2]), ident
            )
        coll = work.tile([P2, 2], FP32, tag="coll")
        scrapV = work.tile([P2, ROWLEN], FP32, tag="scrapV")
        for j in range(2):
            # coll[p] = sum_q (key[q] == key[p]) * ut[p, q]  (later same-cell box)
            nc.vector.scalar_tensor_tensor(
                scrapV[:, 0:P2], qT[j][:], key[:, j : j + 1], ut,
                op0=Alu.is_equal, op1=Alu.mult, accum_out=coll[:, j : j + 1],
            )
        nc.vector.scalar_tensor_tensor(
            stats[:, 6:8], coll[:], 0.0, valid[:], op0=Alu.is_equal, op1=Alu.mult
        )

        # ---- per-(box, anchor) softmax denominators: se = sum_k exp(logit_k)
        # 2 big Exp activations (one per pair-slot) + X-axis reduces, with the
        # label-logit stts interleaved by data arrival
        ex = gpool.tile([P2, 2 * 3 * NC_CLS], FP32, tag="ex")
        for j in range(2):
            gv = graw[:, j * ROWLEN : (j + 1) * ROWLEN].rearrange(
                "p (a f) -> p a f", f=85
            )[:, :, 5:]
            exv = ex[:, j * 3 * NC_CLS : (j + 1) * 3 * NC_CLS].rearrange(
                "p (a f) -> p a f", f=NC_CLS
            )
            nc.scalar.activation(exv, gv, Act.Exp)
            # g3 = sum_k (cidx == cls) * graw  (label-logit sum over 3 anchors)
            nc.vector.scalar_tensor_tensor(
                scrapV[:], cidx, tv[:, j, 0:1],
                graw[:, j * ROWLEN : (j + 1) * ROWLEN],
                op0=Alu.is_equal, op1=Alu.mult, accum_out=stats[:, 8 + j : 9 + j],
            )
            nc.vector.tensor_reduce(
                stats[:, 3 * j : 3 * j + 3], exv, axis=mybir.AxisListType.X,
                op=Alu.add,
            )

        nc.sync.dma_start(out_ap[:], stats[:])


_CACHE = {}


def _get_compiled():
    if "nc" in _CACHE:
        return _CACHE["nc"]
    nc = bacc.Bacc(
        "TRN2",
        target_bir_lowering=False,
        debug=False,
        enable_asserts=False,
        num_devices=N_CORES,
        num_swdge_queues=2,
    )
    x = nc.dram_tensor("xflat", [B_CORE * CELLS, ROWLEN], FP32, kind="ExternalInput")
    t = nc.dram_tensor("tgt", [B_CORE, M, 5], FP32, kind="ExternalInput")
    cpk = nc.dram_tensor("constpk", [P2, NCONST], FP32, kind="ExternalInput")
    out = nc.dram_tensor("statsout", [P2, 10], FP32, kind="ExternalOutput")

    with tile.TileContext(nc) as tc:
        _build_kernel_body(tc, x.ap(), t.ap(), out.ap(), cpk.ap())
    nc.compile()
    _CACHE["nc"] = nc
    return nc


def _finish(stats_list):
    """Host: d = sum_a ln(se) - g3, per-batch mean, global mean (float64)."""
    total = 0.0
    for st in stats_list:
        st = np.asarray(st, dtype=np.float64)  # [100, 10]
        se = st[:, 0:6].reshape(P2, 2, 3)
        win = st[:, 6:8]
        g3 = st[:, 8:10]
        with np.errstate(all="ignore"):
            lnse = np.log(np.maximum(se, 1e-300)).sum(axis=2)
        num = np.where(win > 0.0, lnse - g3, 0.0)
        for j in range(2):
            for bb in range(2):
                sl = slice(bb * M, (bb + 1) * M)
                n = num[sl, j].sum()
                c = win[sl, j].sum()
                total += n / max(3.0 * c, 1.0)
    return total / B


def _run(output, targets, trace=False):
    nc = _get_compiled()
    consts = _host_consts()
    output = np.ascontiguousarray(output, dtype=np.float32)
    targets = np.ascontiguousarray(targets, dtype=np.float32)
    in_maps = []
    for k in range(N_CORES):
        in_maps.append(
            {
                "xflat": output[k * B_CORE : (k + 1) * B_CORE].reshape(
                    B_CORE * CELLS, ROWLEN
                ),
                "tgt": targets[k * B_CORE : (k + 1) * B_CORE],
                **consts,
            }
        )
    res = run_bass_kernel_spmd(nc, in_maps, core_ids=list(range(N_CORES)), trace=trace)
    total = _finish([r["statsout"] for r in res.results])
    return np.float32(total), res


def kernel(output, targets):
    val, _ = _run(output, targets)
    return np.asarray(val, dtype=np.float32)
